# revision 9
# baseline (speedup 1.0000x reference)
"""Trainium2 Bass kernel for nn_ConnectLoss (pairwise BCE-Dice instance loss).

Strategy (8 NeuronCores, pixel-sharded):
  - Each core gets H/8 = 256 rows (524288 pixels) of all four inputs.
  - Heavy part is the joint histogram inter[N=16, K=32] between target/pred
    instance labels. Per core: build fp16 one-hot indicator tiles on DVE
    (tensor_scalar is_equal, 4x mode) and contract 128 pixels/instruction on
    the TensorE into a PSUM-accumulated [16, 32+3] result. The 3 extra moving
    columns carry (cls, ln(cls), ln(1-cls)) so the same matmul also yields
    per-target-class sums needed for the cls_out BCE term.
  - sum(pred_score^2) via ACT Square with accum_out.
  - Marginals sp/st derive from inter row/col sums; tiny final math on host.

cls_out is uniform in [1e-4, 1-1e-4] so the torch-style -100 log clamp can
never trigger; logs are computed unclamped.
"""

import sys

if "/opt/trn_rl_repo" not in sys.path:
    sys.path.insert(0, "/opt/trn_rl_repo")

import numpy as np
from contextlib import ExitStack

# ---------------------------------------------------------------- constants
P = 128
H, W = 2048, 2048
NCORES = 8
ROWS = H // NCORES                 # 256 rows per core
PIX = ROWS * W                     # 524288 pixels per core
FPP = PIX // P                     # 4096 free elems per partition
CF = 1024                          # chunk free size (target one-hots, inputs)
NCHUNK = FPP // CF                 # 4
SCF = 512                          # subchunk free size (pred one-hots + V)
NSUB = CF // SCF                   # 2 subchunks per chunk
K = 32                             # pred instance classes
N = 16                             # target instance classes
NV = 3                             # extra moving cols: cls, ln(cls), ln(1-cls)
MCOL = K + NV                      # 35 moving columns
OUTC = 80                          # output cols: [0:35] inter+aux, [40:72] ps2
NSUBTOT = NCHUNK * NSUB            # 8 subchunks per core
SMOOTH = 1.0
HWPIX = float(H * W)

_cached = {}
TRACE = False


def _build_bass():
    import concourse.bass as bass
    import concourse.bacc as bacc
    import concourse.mybir as mybir
    from concourse.tile import TileContext

    f32 = mybir.dt.float32
    f16 = mybir.dt.float16
    i32 = mybir.dt.int32
    u8 = mybir.dt.uint8
    eq = mybir.AluOpType.is_equal
    AF = mybir.ActivationFunctionType

    nc = bacc.Bacc("TRN2", num_swdge_queues=4)
    pm_d = nc.dram_tensor("pm", [PIX], i32, kind="ExternalInput")
    tm_d = nc.dram_tensor("tm", [PIX], i32, kind="ExternalInput")
    cls_d = nc.dram_tensor("cls", [PIX], f32, kind="ExternalInput")
    ps_d = nc.dram_tensor("ps", [PIX], f32, kind="ExternalInput")
    out_d = nc.dram_tensor("out", [P, OUTC], f32, kind="ExternalOutput")

    pm_v = pm_d[:].rearrange("(p f) -> p f", p=P)
    tm_v = tm_d[:].rearrange("(p f) -> p f", p=P)
    cls_v = cls_d[:].rearrange("(p f) -> p f", p=P)
    ps_v = ps_d[:].rearrange("(p f) -> p f", p=P)

    with ExitStack() as es:
        tc = es.enter_context(TileContext(nc))
        pool_in = es.enter_context(tc.tile_pool(name="inp", bufs=NCHUNK))
        pool_lab = es.enter_context(tc.tile_pool(name="lab", bufs=2))
        pool_toh = es.enter_context(tc.tile_pool(name="toh", bufs=2))
        pool_poh = es.enter_context(tc.tile_pool(name="poh", bufs=2))
        pool_misc = es.enter_context(tc.tile_pool(name="misc", bufs=1))
        pool_scr = es.enter_context(tc.tile_pool(name="scr", bufs=2))
        psum = es.enter_context(tc.tile_pool(name="ps", bufs=1, space="PSUM"))

        inter_ps = psum.tile([N, MCOL], f32)
        ps2acc = pool_misc.tile([P, NSUBTOT], f32)

        for c in range(NCHUNK):
            cs = slice(c * CF, (c + 1) * CF)
            pm_i = pool_in.tile([P, CF], u8, tag="pm_i")
            tm_i = pool_in.tile([P, CF], u8, tag="tm_i")
            cls_t = pool_in.tile([P, CF], f32, tag="cls")
            ps_t = pool_in.tile([P, CF], f32, tag="ps")
            nc.gpsimd.dma_start(out=pm_i[:], in_=pm_v[:, cs])
            nc.gpsimd.dma_start(out=tm_i[:], in_=tm_v[:, cs])
            nc.sync.dma_start(out=cls_t[:], in_=cls_v[:, cs])
            nc.sync.dma_start(out=ps_t[:], in_=ps_v[:, cs])

            pm16 = pool_lab.tile([P, CF], f16, tag="pm16")
            tm16 = pool_lab.tile([P, CF], f16, tag="tm16")
            nc.vector.tensor_copy(pm16[:], pm_i[:])
            nc.vector.tensor_copy(tm16[:], tm_i[:])

            # target one-hots, class-major [P, N*CF]
            toh = pool_toh.tile([P, N * CF], f16, tag="toh")
            for n in range(N):
                nc.vector.tensor_scalar(
                    toh[:, n * CF:(n + 1) * CF], tm16[:], float(n), None, eq
                )

            for s2 in range(NSUB):
                s = c * NSUB + s2
                ss = slice(s2 * SCF, (s2 + 1) * SCF)
                # pred one-hots + V columns, class-major [P, MCOL*SCF]
                pohv = pool_poh.tile([P, MCOL * SCF], f16, tag="pohv")
                for k in range(K):
                    nc.vector.tensor_scalar(
                        pohv[:, k * SCF:(k + 1) * SCF], pm16[:, ss], float(k),
                        None, eq,
                    )
                # V columns on ACT: cls, ln(cls), ln(1-cls)
                nc.scalar.activation(
                    pohv[:, K * SCF:(K + 1) * SCF], cls_t[:, ss], AF.Copy
                )
                nc.scalar.activation(
                    pohv[:, (K + 1) * SCF:(K + 2) * SCF], cls_t[:, ss], AF.Ln
                )
                nc.scalar.activation(
                    pohv[:, (K + 2) * SCF:(K + 3) * SCF], cls_t[:, ss], AF.Ln,
                    bias=1.0, scale=-1.0,
                )
                # sum(pred_score^2) per partition for this subchunk
                scr = pool_scr.tile([P, SCF], f16, tag="scr")
                nc.scalar.activation(
                    scr[:], ps_t[:, ss], AF.Square,
                    accum_out=ps2acc[:, s:s + 1],
                )

                # histogram matmuls: contract 128 pixels per instruction
                toh3 = toh[:].rearrange("p (n f) -> p f n", n=N)
                poh3 = pohv[:].rearrange("p (m f) -> p f m", m=MCOL)
                for jj in range(SCF):
                    j = s2 * SCF + jj
                    first = (c == 0 and j == 0)
                    last = (c == NCHUNK - 1 and j == CF - 1)
                    nc.tensor.matmul(
                        inter_ps[:, :],
                        toh3[:, j:j + 1, :],
                        poh3[:, jj:jj + 1, :],
                        start=first,
                        stop=last,
                    )

        inter_sb = pool_misc.tile([N, MCOL], f32)
        nc.scalar.copy(inter_sb[:], inter_ps[:])
        nc.scalar.dma_start(out=out_d[0:N, 0:MCOL], in_=inter_sb[:])
        nc.scalar.dma_start(out=out_d[:, 40:40 + NSUBTOT], in_=ps2acc[:])

    nc.finalize()
    return nc


def _get_nc():
    if "nc" not in _cached:
        _cached["nc"] = _build_bass()
    return _cached["nc"]


def kernel(pred_instance_mask, pred_score, cls_out, target_mask):
    from concourse.bass_utils import run_bass_kernel_spmd

    nc = _get_nc()

    in_maps = []
    for c in range(NCORES):
        rs = slice(c * ROWS, (c + 1) * ROWS)
        in_maps.append({
            "pm": np.ascontiguousarray(
                pred_instance_mask[rs]).reshape(-1).astype(np.int32),
            "tm": np.ascontiguousarray(
                target_mask[rs]).reshape(-1).astype(np.int32),
            "cls": np.ascontiguousarray(
                cls_out[rs]).reshape(-1).astype(np.float32),
            "ps": np.ascontiguousarray(
                pred_score[rs]).reshape(-1).astype(np.float32),
        })

    res = run_bass_kernel_spmd(
        nc, in_maps, core_ids=list(range(NCORES)), trace=TRACE
    )
    _cached["last_res"] = res
    outs = [r["out"] for r in res.results]

    inter = np.zeros((N, K), dtype=np.float64)
    aux = np.zeros((N, NV), dtype=np.float64)
    ps2 = 0.0
    for o in outs:
        o = o.astype(np.float64)
        inter += o[0:N, 0:K]
        aux += o[0:N, K:K + NV]
        ps2 += o[:, 40:40 + NSUBTOT].sum()

    return _host_finish(inter, aux, ps2)


def _host_finish(inter, aux, ps2):
    st = inter.sum(axis=1)            # [N] target marginals
    sp = inter.sum(axis=0)            # [K] pred marginals
    sum_t = HWPIX - st[0]             # count(target > 0)
    sum_p = aux[:, 0].sum()           # sum(cls_out)
    sum_logp = aux[:, 1].sum()
    inter_cls = sum_p - aux[0, 0]     # sum over target>0 of cls_out
    bce_sum = (sum_logp - aux[0, 1]) + aux[0, 2]

    mse = ps2 / HWPIX
    bce_cls = -bce_sum / HWPIX
    dice_cls = 1.0 - (2.0 * inter_cls + SMOOTH) / (sum_p + sum_t + SMOOTH)

    union = st[:, None] + sp[None, :]
    bce_pair = 100.0 * (union - 2.0 * inter) / HWPIX
    dice_pair = 1.0 - (2.0 * inter + SMOOTH) / (union + SMOOTH)
    pair = bce_pair + dice_pair
    res = mse + bce_cls + dice_cls + pair.min(axis=1).sum()
    return np.float32(res / float(N))


# revision 11
# speedup vs baseline: 35.1930x; 35.1930x over previous
"""Trainium2 Bass kernel for nn_ConnectLoss (pairwise BCE-Dice instance loss).

Strategy (8 NeuronCores, pixel-sharded):
  - Each core gets H/8 = 256 rows (524288 pixels) of all four inputs.
  - Heavy part is the joint histogram inter[N=16, K=32] between target/pred
    instance labels. Per core: build fp16 one-hot indicator tiles on DVE
    (tensor_scalar is_equal, 4x mode) and contract 128 pixels/instruction on
    the TensorE into a PSUM-accumulated [16, 32+3] result. The 3 extra moving
    columns carry (cls, ln(cls), ln(1-cls)) so the same matmul also yields
    per-target-class sums needed for the cls_out BCE term.
  - sum(pred_score^2) via ACT Square with accum_out.
  - Marginals sp/st derive from inter row/col sums; tiny final math on host.

cls_out is uniform in [1e-4, 1-1e-4] so the torch-style -100 log clamp can
never trigger; logs are computed unclamped.
"""

import sys

if "/opt/trn_rl_repo" not in sys.path:
    sys.path.insert(0, "/opt/trn_rl_repo")

import numpy as np
from contextlib import ExitStack

# ---------------------------------------------------------------- constants
P = 128
H, W = 2048, 2048
NCORES = 8
ROWS = H // NCORES                 # 256 rows per core
PIX = ROWS * W                     # 524288 pixels per core
FPP = PIX // P                     # 4096 free elems per partition
CF = 1024                          # chunk free size (target one-hots, inputs)
NCHUNK = FPP // CF                 # 4
SCF = 512                          # subchunk free size (pred one-hots + V)
NSUB = CF // SCF                   # 2 subchunks per chunk
K = 32                             # pred instance classes
N = 16                             # target instance classes
NV = 3                             # extra moving cols: cls, ln(cls), ln(1-cls)
MCOL = K + NV                      # 35 moving columns
OUTC = 80                          # output cols: [0:35] inter+aux, [40:72] ps2
NSUBTOT = NCHUNK * NSUB            # 8 subchunks per core
SMOOTH = 1.0
HWPIX = float(H * W)

_cached = {}
TRACE = False


def _build_bass():
    import concourse.bass as bass
    import concourse.bacc as bacc
    import concourse.mybir as mybir
    from concourse.tile import TileContext

    f32 = mybir.dt.float32
    f16 = mybir.dt.float16
    i32 = mybir.dt.int32
    u8 = mybir.dt.uint8
    eq = mybir.AluOpType.is_equal
    AF = mybir.ActivationFunctionType

    nc = bacc.Bacc("TRN2", num_swdge_queues=4)
    pm_d = nc.dram_tensor("pm", [PIX], i32, kind="ExternalInput")
    tm_d = nc.dram_tensor("tm", [PIX], i32, kind="ExternalInput")
    cls_d = nc.dram_tensor("cls", [PIX], f32, kind="ExternalInput")
    ps_d = nc.dram_tensor("ps", [PIX], f32, kind="ExternalInput")
    out_d = nc.dram_tensor("out", [P, OUTC], f32, kind="ExternalOutput")

    pm_v = pm_d[:].rearrange("(p f) -> p f", p=P)
    tm_v = tm_d[:].rearrange("(p f) -> p f", p=P)
    cls_v = cls_d[:].rearrange("(p f) -> p f", p=P)
    ps_v = ps_d[:].rearrange("(p f) -> p f", p=P)

    with ExitStack() as es:
        tc = es.enter_context(TileContext(nc))
        pool_in = es.enter_context(tc.tile_pool(name="inp", bufs=NCHUNK))
        pool_lab = es.enter_context(tc.tile_pool(name="lab", bufs=2))
        pool_toh = es.enter_context(tc.tile_pool(name="toh", bufs=2))
        pool_poh = es.enter_context(tc.tile_pool(name="poh", bufs=2))
        pool_misc = es.enter_context(tc.tile_pool(name="misc", bufs=1))
        pool_scr = es.enter_context(tc.tile_pool(name="scr", bufs=2))
        psum = es.enter_context(tc.tile_pool(name="ps", bufs=1, space="PSUM"))

        inter_ps = psum.tile([N, MCOL], f32)
        ps2acc = pool_misc.tile([P, NSUBTOT], f32)

        for c in range(NCHUNK):
            cs = slice(c * CF, (c + 1) * CF)
            pm_i = pool_in.tile([P, CF], u8, tag="pm_i")
            tm_i = pool_in.tile([P, CF], u8, tag="tm_i")
            cls_t = pool_in.tile([P, CF], f32, tag="cls")
            ps_t = pool_in.tile([P, CF], f32, tag="ps")
            nc.gpsimd.dma_start(out=pm_i[:], in_=pm_v[:, cs])
            nc.gpsimd.dma_start(out=tm_i[:], in_=tm_v[:, cs])
            nc.sync.dma_start(out=cls_t[:], in_=cls_v[:, cs])
            nc.sync.dma_start(out=ps_t[:], in_=ps_v[:, cs])

            pm16 = pool_lab.tile([P, CF], f16, tag="pm16")
            tm16 = pool_lab.tile([P, CF], f16, tag="tm16")
            nc.vector.tensor_copy(pm16[:], pm_i[:])
            nc.vector.tensor_copy(tm16[:], tm_i[:])

            # target one-hots, class-major [P, N*CF]
            toh = pool_toh.tile([P, N * CF], f16, tag="toh")
            for n in range(N):
                nc.vector.tensor_scalar(
                    toh[:, n * CF:(n + 1) * CF], tm16[:], float(n), None, eq
                )

            for s2 in range(NSUB):
                s = c * NSUB + s2
                ss = slice(s2 * SCF, (s2 + 1) * SCF)
                # pred one-hots + V columns, class-major [P, MCOL*SCF]
                pohv = pool_poh.tile([P, MCOL * SCF], f16, tag="pohv")
                for k in range(K):
                    nc.vector.tensor_scalar(
                        pohv[:, k * SCF:(k + 1) * SCF], pm16[:, ss], float(k),
                        None, eq,
                    )
                # V columns on ACT: cls, ln(cls), ln(1-cls)
                nc.scalar.activation(
                    pohv[:, K * SCF:(K + 1) * SCF], cls_t[:, ss], AF.Copy
                )
                nc.scalar.activation(
                    pohv[:, (K + 1) * SCF:(K + 2) * SCF], cls_t[:, ss], AF.Ln
                )
                nc.scalar.activation(
                    pohv[:, (K + 2) * SCF:(K + 3) * SCF], cls_t[:, ss], AF.Ln,
                    bias=1.0, scale=-1.0,
                )
                # sum(pred_score^2) per partition for this subchunk
                scr = pool_scr.tile([P, SCF], f16, tag="scr")
                nc.scalar.activation(
                    scr[:], ps_t[:, ss], AF.Square,
                    accum_out=ps2acc[:, s:s + 1],
                )

                # histogram matmuls: contract 128 pixels per instruction
                toh3 = toh[:].rearrange("p (n f) -> p f n", n=N)
                poh3 = pohv[:].rearrange("p (m f) -> p f m", m=MCOL)
                for jj in range(SCF):
                    j = s2 * SCF + jj
                    first = (c == 0 and j == 0)
                    last = (c == NCHUNK - 1 and j == CF - 1)
                    nc.tensor.matmul(
                        inter_ps[:, :],
                        toh3[:, j:j + 1, :],
                        poh3[:, jj:jj + 1, :],
                        start=first,
                        stop=last,
                    )

        inter_sb = pool_misc.tile([N, MCOL], f32)
        nc.scalar.copy(inter_sb[:], inter_ps[:])
        nc.scalar.dma_start(out=out_d[0:N, 0:MCOL], in_=inter_sb[:])
        nc.scalar.dma_start(out=out_d[:, 40:40 + NSUBTOT], in_=ps2acc[:])

    nc.finalize()
    return nc


def _get_nc():
    if "nc" not in _cached:
        _cached["nc"] = _build_bass()
    return _cached["nc"]


def _get_runner():
    """Build the sharded jitted executable ONCE; reuse across calls.

    Mirrors concourse.bass2jax.run_bass_via_pjrt's multi-core path, but caches
    the jitted function so repeat calls skip retrace/recompile.
    """
    if "runner" in _cached:
        return _cached["runner"]

    import jax
    import concourse.mybir as mybir
    from jax.sharding import Mesh, PartitionSpec
    from jax.experimental.shard_map import shard_map
    from concourse import bass2jax

    bass2jax.install_neuronx_cc_hook()
    nc = _get_nc()
    partition_name = (
        nc.partition_id_tensor.name if nc.partition_id_tensor else None
    )

    in_names, out_names, out_avals, zero_outs = [], [], [], []
    for alloc in nc.m.functions[0].allocations:
        if not isinstance(alloc, mybir.MemoryLocationSet):
            continue
        name = alloc.memorylocations[0].name
        if alloc.kind == "ExternalInput":
            if name != partition_name:
                in_names.append(name)
        elif alloc.kind == "ExternalOutput":
            out_names.append(name)
            shape = tuple(alloc.tensor_shape)
            dtype = mybir.dt.np(alloc.dtype)
            out_avals.append(jax.core.ShapedArray(shape, dtype))
            zero_outs.append(np.zeros(shape, dtype))
    n_params = len(in_names)
    n_outs = len(out_avals)
    all_in_names = list(in_names) + list(out_names)
    if partition_name is not None:
        all_in_names.append(partition_name)
    donate = tuple(range(n_params, n_params + n_outs))

    def _body(*args):
        operands = list(args)
        if partition_name is not None:
            operands.append(bass2jax.partition_id_tensor())
        outs = bass2jax._bass_exec_p.bind(
            *operands,
            out_avals=tuple(out_avals),
            in_names=tuple(all_in_names),
            out_names=tuple(out_names),
            lowering_input_output_aliases=(),
            sim_require_finite=True,
            sim_require_nnan=True,
            nc=nc,
        )
        return tuple(outs)

    devices = jax.devices()[:NCORES]
    mesh = Mesh(np.asarray(devices), ("core",))
    in_specs = (PartitionSpec("core"),) * (n_params + n_outs)
    out_specs = (PartitionSpec("core"),) * n_outs
    sharded = jax.jit(
        shard_map(
            _body, mesh=mesh, in_specs=in_specs, out_specs=out_specs,
            check_rep=False,
        ),
        donate_argnums=donate,
        keep_unused=True,
    )

    def run(in_maps):
        concat_in = [
            np.concatenate([np.asarray(m[name]) for m in in_maps], axis=0)
            for name in in_names
        ]
        concat_zeros = [
            np.zeros((NCORES * z.shape[0], *z.shape[1:]), z.dtype)
            for z in zero_outs
        ]
        out_arrs = sharded(*concat_in, *concat_zeros)
        return [
            {
                name: np.asarray(out_arrs[i]).reshape(
                    NCORES, *out_avals[i].shape)[c]
                for i, name in enumerate(out_names)
            }
            for c in range(NCORES)
        ]

    def bench(in_maps, iters=20):
        """Time the sharded call with device-resident inputs."""
        import time
        from jax.sharding import NamedSharding

        concat_in = [
            np.concatenate([np.asarray(m[name]) for m in in_maps], axis=0)
            for name in in_names
        ]
        shard = NamedSharding(mesh, PartitionSpec("core"))
        dev_in = [jax.device_put(x, shard) for x in concat_in]
        zeros = [
            np.zeros((NCORES * z.shape[0], *z.shape[1:]), z.dtype)
            for z in zero_outs
        ]

        def call():
            zs = [jax.device_put(z, shard) for z in zeros]
            outs = sharded(*dev_in, *zs)
            for o in outs:
                o.block_until_ready()

        call()
        ts = []
        for _ in range(iters):
            t0 = time.perf_counter()
            call()
            ts.append(time.perf_counter() - t0)
        return min(ts), sum(ts) / len(ts)

    run.bench = bench
    _cached["runner"] = run
    return run


def kernel(pred_instance_mask, pred_score, cls_out, target_mask):
    run = _get_runner()

    in_maps = []
    for c in range(NCORES):
        rs = slice(c * ROWS, (c + 1) * ROWS)
        in_maps.append({
            "pm": np.ascontiguousarray(
                pred_instance_mask[rs]).reshape(-1).astype(np.int32),
            "tm": np.ascontiguousarray(
                target_mask[rs]).reshape(-1).astype(np.int32),
            "cls": np.ascontiguousarray(
                cls_out[rs]).reshape(-1).astype(np.float32),
            "ps": np.ascontiguousarray(
                pred_score[rs]).reshape(-1).astype(np.float32),
        })

    outs = [r["out"] for r in run(in_maps)]

    inter = np.zeros((N, K), dtype=np.float64)
    aux = np.zeros((N, NV), dtype=np.float64)
    ps2 = 0.0
    for o in outs:
        o = o.astype(np.float64)
        inter += o[0:N, 0:K]
        aux += o[0:N, K:K + NV]
        ps2 += o[:, 40:40 + NSUBTOT].sum()

    return _host_finish(inter, aux, ps2)


def _host_finish(inter, aux, ps2):
    st = inter.sum(axis=1)            # [N] target marginals
    sp = inter.sum(axis=0)            # [K] pred marginals
    sum_t = HWPIX - st[0]             # count(target > 0)
    sum_p = aux[:, 0].sum()           # sum(cls_out)
    sum_logp = aux[:, 1].sum()
    inter_cls = sum_p - aux[0, 0]     # sum over target>0 of cls_out
    bce_sum = (sum_logp - aux[0, 1]) + aux[0, 2]

    mse = ps2 / HWPIX
    bce_cls = -bce_sum / HWPIX
    dice_cls = 1.0 - (2.0 * inter_cls + SMOOTH) / (sum_p + sum_t + SMOOTH)

    union = st[:, None] + sp[None, :]
    bce_pair = 100.0 * (union - 2.0 * inter) / HWPIX
    dice_pair = 1.0 - (2.0 * inter + SMOOTH) / (union + SMOOTH)
    pair = bce_pair + dice_pair
    res = mse + bce_cls + dice_cls + pair.min(axis=1).sum()
    return np.float32(res / float(N))


# revision 15
# speedup vs baseline: 658.8559x; 18.7212x over previous
"""Trainium2 Bass kernel for nn_ConnectLoss (pairwise BCE-Dice instance loss).

Strategy (8 NeuronCores, pixel-sharded):
  - Each core gets H/8 = 256 rows (524288 pixels) of all four inputs.
  - Heavy part is the joint histogram inter[N=16, K=32] between target/pred
    instance labels. Per core: build fp16 one-hot indicator tiles on DVE
    (tensor_scalar is_equal, 4x mode) and contract 128 pixels/instruction on
    the TensorE into a PSUM-accumulated [16, 32+3] result. The 3 extra moving
    columns carry (cls, ln(cls), ln(1-cls)) so the same matmul also yields
    per-target-class sums needed for the cls_out BCE term.
  - sum(pred_score^2) via ACT Square with accum_out.
  - Marginals sp/st derive from inter row/col sums; tiny final math on host.

cls_out is uniform in [1e-4, 1-1e-4] so the torch-style -100 log clamp can
never trigger; logs are computed unclamped.
"""

import sys

if "/opt/trn_rl_repo" not in sys.path:
    sys.path.insert(0, "/opt/trn_rl_repo")

import numpy as np
from contextlib import ExitStack

# ---------------------------------------------------------------- constants
P = 128
H, W = 2048, 2048
NCORES = 8
ROWS = H // NCORES                 # 256 rows per core
PIX = ROWS * W                     # 524288 pixels per core
FPP = PIX // P                     # 4096 free elems per partition
CF = 1024                          # chunk free size (target one-hots, inputs)
NCHUNK = FPP // CF                 # 4
SCF = 512                          # subchunk free size (pred one-hots + V)
NSUB = CF // SCF                   # 2 subchunks per chunk
K = 32                             # pred instance classes
N = 16                             # target instance classes
NV = 3                             # extra moving cols: cls, ln(cls), ln(1-cls)
MCOL = K + NV                      # 35 moving columns
OUTC = 80                          # output cols: [0:35] inter+aux, [40:72] ps2
NSUBTOT = NCHUNK * NSUB            # 8 subchunks per core
SMOOTH = 1.0
HWPIX = float(H * W)

_cached = {}
TRACE = False


def _build_bass():
    import concourse.bass as bass
    import concourse.bacc as bacc
    import concourse.mybir as mybir
    from concourse.tile import TileContext

    f32 = mybir.dt.float32
    f16 = mybir.dt.float16
    i32 = mybir.dt.int32
    u8 = mybir.dt.uint8
    eq = mybir.AluOpType.is_equal
    AF = mybir.ActivationFunctionType

    nc = bacc.Bacc("TRN2", num_swdge_queues=4)
    pm_d = nc.dram_tensor("pm", [PIX], i32, kind="ExternalInput")
    tm_d = nc.dram_tensor("tm", [PIX], i32, kind="ExternalInput")
    cls_d = nc.dram_tensor("cls", [PIX], f32, kind="ExternalInput")
    ps_d = nc.dram_tensor("ps", [PIX], f32, kind="ExternalInput")
    out_d = nc.dram_tensor("out", [P, OUTC], f32, kind="ExternalOutput")

    pm_v = pm_d[:].rearrange("(p f) -> p f", p=P)
    tm_v = tm_d[:].rearrange("(p f) -> p f", p=P)
    cls_v = cls_d[:].rearrange("(p f) -> p f", p=P)
    ps_v = ps_d[:].rearrange("(p f) -> p f", p=P)

    with ExitStack() as es:
        tc = es.enter_context(TileContext(nc))
        pool_in = es.enter_context(tc.tile_pool(name="inp", bufs=2))
        pool_toh = es.enter_context(tc.tile_pool(name="toh", bufs=2))
        pool_poh = es.enter_context(tc.tile_pool(name="poh", bufs=2))
        pool_misc = es.enter_context(tc.tile_pool(name="misc", bufs=1))
        pool_scr = es.enter_context(tc.tile_pool(name="scr", bufs=2))
        psum = es.enter_context(tc.tile_pool(name="ps", bufs=1, space="PSUM"))

        inter_ps = psum.tile([N, MCOL], f32)
        ps2acc = pool_misc.tile([P, NCHUNK], f32)

        for c in range(NCHUNK):
            cs = slice(c * CF, (c + 1) * CF)
            # labels arrive as fp16 via casting DMA (gpsimd SWDGE)
            pm16 = pool_in.tile([P, CF], f16, tag="pm16")
            tm16 = pool_in.tile([P, CF], f16, tag="tm16")
            cls_t = pool_in.tile([P, CF], f32, tag="cls")
            ps_t = pool_in.tile([P, CF], f16, tag="ps")
            nc.gpsimd.dma_start(out=pm16[:], in_=pm_v[:, cs])
            nc.gpsimd.dma_start(out=tm16[:], in_=tm_v[:, cs])
            nc.sync.dma_start(out=cls_t[:], in_=cls_v[:, cs])
            nc.gpsimd.dma_start(out=ps_t[:], in_=ps_v[:, cs])

            # pred one-hots + V columns at CF granularity, class-major
            pohv = pool_poh.tile([P, MCOL * CF], f16, tag="pohv")
            for k in range(K):
                nc.vector.tensor_scalar(
                    pohv[:, k * CF:(k + 1) * CF], pm16[:], float(k), None, eq
                )
            # V columns on ACT: cls, ln(cls), ln(1-cls)
            nc.scalar.activation(
                pohv[:, K * CF:(K + 1) * CF], cls_t[:], AF.Copy
            )
            nc.scalar.activation(
                pohv[:, (K + 1) * CF:(K + 2) * CF], cls_t[:], AF.Ln
            )
            nc.scalar.activation(
                pohv[:, (K + 2) * CF:(K + 3) * CF], cls_t[:], AF.Ln,
                bias=1.0, scale=-1.0,
            )
            # sum(pred_score^2) per partition for this chunk
            scr = pool_scr.tile([P, CF], f16, tag="scr")
            nc.scalar.activation(
                scr[:], ps_t[:], AF.Square,
                accum_out=ps2acc[:, c:c + 1],
            )

            poh3 = pohv[:].rearrange("p (m f) -> p f m", m=MCOL)
            for s2 in range(NSUB):
                # target one-hots at SCF granularity, class-major
                toh = pool_toh.tile([P, N * SCF], f16, tag="toh")
                for n in range(N):
                    nc.vector.tensor_scalar(
                        toh[:, n * SCF:(n + 1) * SCF],
                        tm16[:, s2 * SCF:(s2 + 1) * SCF], float(n), None, eq,
                    )
                toh3 = toh[:].rearrange("p (n f) -> p f n", n=N)
                for jj in range(SCF):
                    j = s2 * SCF + jj
                    first = (c == 0 and j == 0)
                    last = (c == NCHUNK - 1 and j == CF - 1)
                    nc.tensor.matmul(
                        inter_ps[:, :],
                        toh3[:, jj:jj + 1, :],
                        poh3[:, j:j + 1, :],
                        start=first,
                        stop=last,
                    )

        inter_sb = pool_misc.tile([N, MCOL], f32)
        nc.scalar.copy(inter_sb[:], inter_ps[:])
        nc.scalar.dma_start(out=out_d[0:N, 0:MCOL], in_=inter_sb[:])
        nc.scalar.dma_start(out=out_d[:, 40:40 + NCHUNK], in_=ps2acc[:])

    nc.finalize()
    return nc


def _get_nc():
    if "nc" not in _cached:
        _cached["nc"] = _build_bass()
    return _cached["nc"]


def _get_runner():
    """Build the sharded jitted executable ONCE; reuse across calls.

    Mirrors concourse.bass2jax.run_bass_via_pjrt's multi-core path, but caches
    the jitted function so repeat calls skip retrace/recompile.
    """
    if "runner" in _cached:
        return _cached["runner"]

    import jax
    import concourse.mybir as mybir
    from jax.sharding import Mesh, PartitionSpec
    from jax.experimental.shard_map import shard_map
    from concourse import bass2jax

    bass2jax.install_neuronx_cc_hook()
    nc = _get_nc()
    partition_name = (
        nc.partition_id_tensor.name if nc.partition_id_tensor else None
    )

    in_names, out_names, out_avals, zero_outs = [], [], [], []
    for alloc in nc.m.functions[0].allocations:
        if not isinstance(alloc, mybir.MemoryLocationSet):
            continue
        name = alloc.memorylocations[0].name
        if alloc.kind == "ExternalInput":
            if name != partition_name:
                in_names.append(name)
        elif alloc.kind == "ExternalOutput":
            out_names.append(name)
            shape = tuple(alloc.tensor_shape)
            dtype = mybir.dt.np(alloc.dtype)
            out_avals.append(jax.core.ShapedArray(shape, dtype))
            zero_outs.append(np.zeros(shape, dtype))
    n_params = len(in_names)
    n_outs = len(out_avals)
    all_in_names = list(in_names) + list(out_names)
    if partition_name is not None:
        all_in_names.append(partition_name)
    donate = tuple(range(n_params, n_params + n_outs))

    def _body(*args):
        operands = list(args)
        if partition_name is not None:
            operands.append(bass2jax.partition_id_tensor())
        outs = bass2jax._bass_exec_p.bind(
            *operands,
            out_avals=tuple(out_avals),
            in_names=tuple(all_in_names),
            out_names=tuple(out_names),
            lowering_input_output_aliases=(),
            sim_require_finite=True,
            sim_require_nnan=True,
            nc=nc,
        )
        return tuple(outs)

    devices = jax.devices()[:NCORES]
    mesh = Mesh(np.asarray(devices), ("core",))
    in_specs = (PartitionSpec("core"),) * (n_params + n_outs)
    out_specs = (PartitionSpec("core"),) * n_outs
    sharded = jax.jit(
        shard_map(
            _body, mesh=mesh, in_specs=in_specs, out_specs=out_specs,
            check_rep=False,
        ),
        donate_argnums=donate,
        keep_unused=True,
    )

    def run(in_maps):
        concat_in = [
            np.concatenate([np.asarray(m[name]) for m in in_maps], axis=0)
            for name in in_names
        ]
        concat_zeros = [
            np.zeros((NCORES * z.shape[0], *z.shape[1:]), z.dtype)
            for z in zero_outs
        ]
        out_arrs = sharded(*concat_in, *concat_zeros)
        return [
            {
                name: np.asarray(out_arrs[i]).reshape(
                    NCORES, *out_avals[i].shape)[c]
                for i, name in enumerate(out_names)
            }
            for c in range(NCORES)
        ]

    def bench(in_maps, iters=20):
        """Time the sharded call with device-resident inputs."""
        import time
        from jax.sharding import NamedSharding

        concat_in = [
            np.concatenate([np.asarray(m[name]) for m in in_maps], axis=0)
            for name in in_names
        ]
        shard = NamedSharding(mesh, PartitionSpec("core"))
        dev_in = [jax.device_put(x, shard) for x in concat_in]
        zeros = [
            np.zeros((NCORES * z.shape[0], *z.shape[1:]), z.dtype)
            for z in zero_outs
        ]

        def call():
            zs = [jax.device_put(z, shard) for z in zeros]
            outs = sharded(*dev_in, *zs)
            for o in outs:
                o.block_until_ready()

        call()
        ts = []
        for _ in range(iters):
            t0 = time.perf_counter()
            call()
            ts.append(time.perf_counter() - t0)
        return min(ts), sum(ts) / len(ts)

    run.bench = bench
    _cached["runner"] = run
    return run


def kernel(pred_instance_mask, pred_score, cls_out, target_mask):
    run = _get_runner()

    in_maps = []
    for c in range(NCORES):
        rs = slice(c * ROWS, (c + 1) * ROWS)
        in_maps.append({
            "pm": np.ascontiguousarray(
                pred_instance_mask[rs]).reshape(-1).astype(np.int32),
            "tm": np.ascontiguousarray(
                target_mask[rs]).reshape(-1).astype(np.int32),
            "cls": np.ascontiguousarray(
                cls_out[rs]).reshape(-1).astype(np.float32),
            "ps": np.ascontiguousarray(
                pred_score[rs]).reshape(-1).astype(np.float32),
        })

    outs = [r["out"] for r in run(in_maps)]

    inter = np.zeros((N, K), dtype=np.float64)
    aux = np.zeros((N, NV), dtype=np.float64)
    ps2 = 0.0
    for o in outs:
        o = o.astype(np.float64)
        inter += o[0:N, 0:K]
        aux += o[0:N, K:K + NV]
        ps2 += o[:, 40:40 + NCHUNK].sum()

    return _host_finish(inter, aux, ps2)


def _host_finish(inter, aux, ps2):
    st = inter.sum(axis=1)            # [N] target marginals
    sp = inter.sum(axis=0)            # [K] pred marginals
    sum_t = HWPIX - st[0]             # count(target > 0)
    sum_p = aux[:, 0].sum()           # sum(cls_out)
    sum_logp = aux[:, 1].sum()
    inter_cls = sum_p - aux[0, 0]     # sum over target>0 of cls_out
    bce_sum = (sum_logp - aux[0, 1]) + aux[0, 2]

    mse = ps2 / HWPIX
    bce_cls = -bce_sum / HWPIX
    dice_cls = 1.0 - (2.0 * inter_cls + SMOOTH) / (sum_p + sum_t + SMOOTH)

    union = st[:, None] + sp[None, :]
    bce_pair = 100.0 * (union - 2.0 * inter) / HWPIX
    dice_pair = 1.0 - (2.0 * inter + SMOOTH) / (union + SMOOTH)
    pair = bce_pair + dice_pair
    res = mse + bce_cls + dice_cls + pair.min(axis=1).sum()
    return np.float32(res / float(N))
